# revision 14
# baseline (speedup 1.0000x reference)
"""Trainium2 Bass kernel for nn_DecodingLoss_BCEBased (BP decoding BCE loss).

Math: per check m with support S_m (|S_m|=6) and per (b,t):
    p = prod_{j in S_m} tanh(llr_j/2);  z = -2 arctanh(p);  y = syndrome
    BCE(z, y) = log2 - log(1 + s*p),  s = 1 - 2y
so  loss = 0.5*(M+K)*log2 - 0.5 * sum_{b,t,m} log(1 + s*p) / (B*T)
(identically for the K observables with |S|=50).

Device strategy (pure data parallel over batch, 16 batches/core):
  - host pre-transposes llrs to column-major [N, R] per core (R = T*16 rows,
    t-major so row r has batch b = r%16), pads N->79*128 and R->512, and
    appends a llr=88 column whose tanh is exactly 1.0 (neutral pad element).
  - device: DMA llrsT -> ACT tanh -> t bf16 in SBUF, N-major (col j at
    partition j%128, rank j//128).
  - gpsimd dma_gather (SBUF source, transpose mode) fetches the 6 support
    columns per check, w-major in chunks of 640 checks: G[p, f, w, c] with
    row r = f*128+p on partitions.
  - DVE product tree (in-place in G) + per-(row,check) sign multiply,
    ACT Ln(x+1) with fused free-dim sum accumulation.
  - ones-matmul partition reduce + affine -> per-core scalar; host sums the
    8 per-core scalars.
"""

import math
import os

import numpy as np

try:
    import concourse.bass as bass  # noqa: F401
except ImportError:  # pragma: no cover
    import sys

    sys.path.insert(0, "/opt/trn_rl_repo")

import ml_dtypes
import concourse.bacc as bacc
import concourse.bass as bass
import concourse.mybir as mybir
import concourse.tile as tile
from concourse.bass_utils import run_bass_kernel_spmd

F32 = mybir.dt.float32
BF16 = mybir.dt.bfloat16
I16 = mybir.dt.int16
AF = mybir.ActivationFunctionType

# ---- problem constants (hardcoded per spec) ----
B, T, N, M, K = 128, 30, 10000, 5000, 10
CHK_W, OBS_W = 6, 50
NCORES = 8
BC = B // NCORES            # 16 batches per core
R = T * BC                  # 480 real rows per core
RP = 512                    # padded rows (gather elem_size)
FD = RP // 128              # 4
NRANKS = (N + 128) // 128   # 79 (includes room for the ones column)
NPAD = NRANKS * 128         # 10112
ONES_COL = NPAD - 1         # llr=88 -> tanh==1.0 (neutral for products)
CPC = 640                   # checks per gather chunk
NCHUNK = 8                  # 5120 padded checks
MPAD = NCHUNK * CPC
EPC = CPC * CHK_W           # 3840 gather idxs per chunk (%128 == 0)
OWPAD = 64                  # obs support padded to 64 (halving tree)
EOBS = OWPAD * K            # 640 (%128 == 0)
EPG = 768                   # idxs per dma_gather (descriptor ring cap 1023)
GPC = EPC // EPG            # 5 gathers per compute chunk
CPG = EPG // CHK_W          # 128 checks per gather
RANK_CHUNK = 16             # llr load/tanh staging granularity

_CONST = 0.5 * (M + K) * math.log(2.0) / NCORES
_SCALE = -0.5 / (B * T)

_nc_cache = None
LAST_RESULTS = None  # BassKernelResults from the most recent run (debug)


def _build_nc(nchunk=NCHUNK, do_muls=True, do_obs=True, do_final=True):
    nc = bacc.Bacc(
        "TRN2", target_bir_lowering=False, debug=False, enable_asserts=True
    )
    llrs_t = nc.dram_tensor("llrs_t", [NPAD, RP], F32, kind="ExternalInput")
    idx_syn = nc.dram_tensor(
        "idx_syn", [128, NCHUNK * (EPC // 16)], I16, kind="ExternalInput"
    )
    idx_obs = nc.dram_tensor("idx_obs", [128, EOBS // 16], I16, kind="ExternalInput")
    sign_syn = nc.dram_tensor("sign_syn", [128, MPAD], BF16, kind="ExternalInput")
    sign_obs = nc.dram_tensor("sign_obs", [128, K], BF16, kind="ExternalInput")
    out = nc.dram_tensor("out", [1, 1], F32, kind="ExternalOutput")

    with tile.TileContext(nc) as tc:
        import contextlib

        with contextlib.ExitStack() as ctx:
            persist = ctx.enter_context(tc.tile_pool(name="persist", bufs=1))
            small = ctx.enter_context(tc.tile_pool(name="small", bufs=1))
            psum = ctx.enter_context(tc.tile_pool(name="ps", bufs=1, space="PSUM"))

            idx_syn_sb = persist.tile([128, NCHUNK * (EPC // 16)], I16)
            nc.sync.dma_start(idx_syn_sb[:], idx_syn[:])
            idx_obs_sb = persist.tile([128, EOBS // 16], I16)
            nc.sync.dma_start(idx_obs_sb[:], idx_obs[:])
            sign_syn_sb = persist.tile([128, MPAD], BF16)
            nc.sync.dma_start(sign_syn_sb[:], sign_syn[:])
            sign_obs_sb = persist.tile([128, K], BF16)
            nc.sync.dma_start(sign_obs_sb[:], sign_obs[:])

            # t (bf16, N-major): col j at partition j%128, rank j//128
            t_sb = persist.tile([128, NRANKS, RP], BF16)
            llr_src = llrs_t[:].rearrange("(a p) r -> p a r", p=128)
            with tc.tile_pool(name="llr_stage", bufs=2) as stage:
                for a0 in range(0, NRANKS, RANK_CHUNK):
                    a1 = min(a0 + RANK_CHUNK, NRANKS)
                    st = stage.tile([128, RANK_CHUNK, RP], F32, tag="stage")
                    nc.sync.dma_start(
                        st[:, : a1 - a0, :], llr_src[:, a0:a1, :]
                    )
                    nc.scalar.activation(
                        t_sb[:, a0:a1, :], st[:, : a1 - a0, :], AF.Tanh, scale=0.5
                    )

            acc = small.tile([128, NCHUNK + 1], F32)

            with tc.tile_pool(name="g", bufs=2) as gpool:
                for c in range(nchunk):
                    # SWDGE descriptor ring holds 1023 descs/direction: a
                    # >1023-idx dma_gather wedges the Q7. Issue EPG=768-idx
                    # gathers, each into its own contiguous [:, k] block
                    # (transpose-mode output must be contiguous).
                    g = gpool.tile([128, GPC, FD, CHK_W * CPG], BF16, tag="G")
                    for k in range(GPC):
                        nc.gpsimd.dma_gather(
                            g[:, k],
                            t_sb[:],
                            idx_syn_sb[
                                :,
                                c * (EPC // 16) + k * (EPG // 16) : c * (EPC // 16)
                                + (k + 1) * (EPG // 16),
                            ],
                            EPG,
                            EPG,
                            RP,
                            transpose=True,
                            sbuf_tokens_per_rank=128,
                            sbuf_free_dim_per_rank=RP * 2,
                            sbuf_free_dim_pad_per_rank=0,
                            sbuf_byte_offset=0,
                        )
                    gv = g[:].rearrange("p k f (w c) -> p k f w c", w=CHK_W)
                    if not do_muls:
                        nc.scalar.activation(
                            gv[:, :, :, 0], gv[:, :, :, 0], AF.Ln, bias=1.0,
                            accum_out=acc[:, c:c + 1])
                        continue
                    nc.vector.tensor_mul(gv[:, :, :, 1], gv[:, :, :, 0], gv[:, :, :, 1])
                    nc.vector.tensor_mul(gv[:, :, :, 3], gv[:, :, :, 2], gv[:, :, :, 3])
                    nc.vector.tensor_mul(gv[:, :, :, 5], gv[:, :, :, 4], gv[:, :, :, 5])
                    nc.vector.tensor_mul(gv[:, :, :, 3], gv[:, :, :, 1], gv[:, :, :, 3])
                    sgn = sign_syn_sb[:, c * CPC : (c + 1) * CPC].rearrange(
                        "p (k c) -> p k c", k=GPC
                    )
                    for f in range(FD):
                        nc.vector.tensor_mul(gv[:, :, f, 5], gv[:, :, f, 5], sgn)
                    nc.vector.tensor_mul(gv[:, :, :, 0], gv[:, :, :, 3], gv[:, :, :, 5])
                    nc.scalar.activation(
                        gv[:, :, :, 0],
                        gv[:, :, :, 0],
                        AF.Ln,
                        bias=1.0,
                        accum_out=acc[:, c : c + 1],
                    )

                # observables: one chunk, halving tree over w (pads are 1.0)
                if do_obs:
                    g = gpool.tile([128, FD, EOBS], BF16, tag="G")
                    nc.gpsimd.dma_gather(
                        g[:],
                        t_sb[:],
                        idx_obs_sb[:],
                        EOBS,
                        EOBS,
                        RP,
                        transpose=True,
                        sbuf_tokens_per_rank=128,
                        sbuf_free_dim_per_rank=RP * 2,
                        sbuf_free_dim_pad_per_rank=0,
                        sbuf_byte_offset=0,
                    )
                    gv = g[:].rearrange("p f (w k) -> p f w k", w=OWPAD)
                    half = OWPAD // 2
                    while half >= 1:
                        nc.vector.tensor_mul(
                            gv[:, :, 0:half], gv[:, :, 0:half],
                            gv[:, :, half : 2 * half]
                        )
                        half //= 2
                    for f in range(FD):
                        nc.vector.tensor_mul(gv[:, f, 0], gv[:, f, 0],
                                             sign_obs_sb[:])
                    nc.scalar.activation(
                        gv[:, :, 0],
                        gv[:, :, 0],
                        AF.Ln,
                        bias=1.0,
                        accum_out=acc[:, NCHUNK : NCHUNK + 1],
                    )
                else:
                    nc.vector.memset(acc[:, NCHUNK : NCHUNK + 1], 0.0)

            if do_final:
                accsum = small.tile([128, 1], F32)
                nc.vector.tensor_reduce(
                    accsum[:], acc[:], mybir.AxisListType.X, mybir.AluOpType.add
                )
                ones = small.tile([128, 1], F32)
                nc.vector.memset(ones[:], 1.0)
                tot_ps = psum.tile([1, 1], F32)
                nc.tensor.matmul(tot_ps[:], ones[:], accsum[:])
                res = small.tile([1, 1], F32)
                cbias = small.tile([1, 1], F32)
                nc.vector.memset(cbias[:], _CONST)
                nc.scalar.activation(
                    res[:], tot_ps[:], AF.Identity, scale=_SCALE, bias=cbias[:]
                )
                nc.sync.dma_start(out[:], res[:])
            else:
                res = small.tile([1, 1], F32)
                nc.vector.tensor_reduce(
                    res[:], acc[:1, :], mybir.AxisListType.X, mybir.AluOpType.add
                )
                nc.sync.dma_start(out[:], res[:])

    nc.compile()
    return nc


def _segments(idx, seg, nseg, width):
    """Group entry column-indices by segment id -> [nseg, width] int array."""
    idx = np.asarray(idx, np.int64).ravel()
    seg = np.asarray(seg, np.int64).ravel()
    counts = np.bincount(seg, minlength=nseg)
    assert counts.max() <= width, (counts.max(), width)
    order = np.argsort(seg, kind="stable")
    out = np.full((nseg, width), ONES_COL, np.int64)
    srt = idx[order]
    pos = 0
    if (counts == width).all():
        out[:, :] = srt.reshape(nseg, width)
    else:
        for s in range(nseg):
            c = counts[s]
            out[s, :c] = srt[pos : pos + c]
            pos += c
    return out


def _wrap_idxs(flat):
    """[n] -> [128, n//16] int16, gather pos i = s*16 + (p%16)."""
    n = len(flat)
    s = np.arange(n // 16)
    out = np.empty((128, n // 16), np.int16)
    for p in range(128):
        out[p, :] = flat[s * 16 + (p % 16)]
    return out


def _prepare_in_maps(all_llrs, syndromes, observables, chk_idx, chk_seg,
                     obs_idx, obs_seg):
    all_llrs = np.asarray(all_llrs, np.float32)
    syndromes = np.asarray(syndromes)
    observables = np.asarray(observables)

    e6 = _segments(chk_idx, chk_seg, M, CHK_W)          # [M, 6]
    e6 = np.concatenate(
        [e6, np.full((MPAD - M, CHK_W), ONES_COL, np.int64)], axis=0
    )
    eo = _segments(obs_idx, obs_seg, K, OBS_W)          # [K, 50]
    eo = np.concatenate(
        [eo, np.full((K, OWPAD - OBS_W), ONES_COL, np.int64)], axis=1
    )

    # gather index tables (identical for every core)
    idx_syn = np.empty((128, NCHUNK * (EPC // 16)), np.int16)
    for c in range(NCHUNK):
        flat = np.empty(EPC, np.int64)
        for k in range(GPC):
            for w in range(CHK_W):
                lo = c * CPC + k * CPG
                flat[k * EPG + w * CPG : k * EPG + (w + 1) * CPG] = e6[
                    lo : lo + CPG, w
                ]
        idx_syn[:, c * (EPC // 16) : (c + 1) * (EPC // 16)] = _wrap_idxs(flat)
    flat = np.empty(EOBS, np.int64)
    for w in range(OWPAD):
        flat[w * K : (w + 1) * K] = eo[:, w]
    idx_obs = _wrap_idxs(flat)

    p16 = np.arange(128) % 16
    in_maps = []
    for core in range(NCORES):
        b0 = core * BC
        lt = np.zeros((NPAD, RP), np.float32)
        # row r = t*16 + b  (t-major)
        lt[:N, :R] = all_llrs[b0 : b0 + BC].transpose(2, 1, 0).reshape(N, R)
        lt[ONES_COL, :] = 88.0
        ss = np.zeros((128, MPAD), np.float32)
        ss[:, :M] = 1.0 - 2.0 * syndromes[b0 + p16, :].astype(np.float32)
        so = 1.0 - 2.0 * observables[b0 + p16, :].astype(np.float32)
        in_maps.append(
            {
                "llrs_t": lt,
                "idx_syn": idx_syn,
                "idx_obs": idx_obs,
                "sign_syn": ss.astype(ml_dtypes.bfloat16),
                "sign_obs": so.astype(ml_dtypes.bfloat16),
            }
        )
    return in_maps


def kernel(all_llrs, syndromes, observables, chk_idx, chk_seg, obs_idx, obs_seg):
    global _nc_cache, LAST_RESULTS
    in_maps = _prepare_in_maps(
        all_llrs, syndromes, observables, chk_idx, chk_seg, obs_idx, obs_seg
    )

    if _nc_cache is None:
        _nc_cache = _build_nc()

    trace = os.environ.get("KERNEL_TRACE", "") == "1"
    LAST_RESULTS = run_bass_kernel_spmd(
        _nc_cache, in_maps, core_ids=list(range(NCORES)), trace=trace
    )
    total = sum(float(r["out"][0, 0]) for r in LAST_RESULTS.results)
    return np.float32(total)


# revision 15
# speedup vs baseline: 1.0163x; 1.0163x over previous
"""Trainium2 Bass kernel for nn_DecodingLoss_BCEBased (BP decoding BCE loss).

Math: per check m with support S_m (|S_m|=6) and per (b,t):
    p = prod_{j in S_m} tanh(llr_j/2);  z = -2 arctanh(p);  y = syndrome
    BCE(z, y) = log2 - log(1 + s*p),  s = 1 - 2y
so  loss = 0.5*(M+K)*log2 - 0.5 * sum_{b,t,m} log(1 + s*p) / (B*T)
(identically for the K observables with |S|=50).

Device strategy (pure data parallel over batch, 16 batches/core):
  - host pre-transposes llrs to column-major [N, R] per core (R = T*16 rows,
    t-major so row r has batch b = r%16), pads N->79*128 and R->512, and
    appends a llr=88 column whose tanh is exactly 1.0 (neutral pad element).
  - device: DMA llrsT -> ACT tanh -> t bf16 in SBUF, N-major (col j at
    partition j%128, rank j//128).
  - gpsimd dma_gather (SBUF source, transpose mode) fetches the 6 support
    columns per check, w-major in chunks of 640 checks: G[p, f, w, c] with
    row r = f*128+p on partitions.
  - DVE product tree (in-place in G) + per-(row,check) sign multiply,
    ACT Ln(x+1) with fused free-dim sum accumulation.
  - ones-matmul partition reduce + affine -> per-core scalar; host sums the
    8 per-core scalars.
"""

import math
import os

import numpy as np

try:
    import concourse.bass as bass  # noqa: F401
except ImportError:  # pragma: no cover
    import sys

    sys.path.insert(0, "/opt/trn_rl_repo")

import ml_dtypes
import concourse.bacc as bacc
import concourse.bass as bass
import concourse.mybir as mybir
import concourse.tile as tile
from concourse.bass_utils import run_bass_kernel_spmd

F32 = mybir.dt.float32
BF16 = mybir.dt.bfloat16
I16 = mybir.dt.int16
AF = mybir.ActivationFunctionType

# ---- problem constants (hardcoded per spec) ----
B, T, N, M, K = 128, 30, 10000, 5000, 10
CHK_W, OBS_W = 6, 50
NCORES = 8
BC = B // NCORES            # 16 batches per core
R = T * BC                  # 480 real rows per core
RP = 512                    # padded rows (gather elem_size)
FD = RP // 128              # 4
NRANKS = (N + 128) // 128   # 79 (includes room for the ones column)
NPAD = NRANKS * 128         # 10112
ONES_COL = NPAD - 1         # llr=88 -> tanh==1.0 (neutral for products)
CPC = 640                   # checks per gather chunk
NCHUNK = 8                  # 5120 padded checks
MPAD = NCHUNK * CPC
EPC = CPC * CHK_W           # 3840 gather idxs per chunk (%128 == 0)
OWPAD = 64                  # obs support padded to 64 (halving tree)
EOBS = OWPAD * K            # 640 (%128 == 0)
EPG = 768                   # idxs per dma_gather (descriptor ring cap 1023)
GPC = EPC // EPG            # 5 gathers per compute chunk
CPG = EPG // CHK_W          # 128 checks per gather
RANK_CHUNK = 16             # llr load/tanh staging granularity

_CONST = 0.5 * (M + K) * math.log(2.0) / NCORES
_SCALE = -0.5 / (B * T)

_nc_cache = None
LAST_RESULTS = None  # BassKernelResults from the most recent run (debug)


def _build_nc(nchunk=NCHUNK, do_muls=True, do_obs=True, do_final=True):
    nc = bacc.Bacc(
        "TRN2",
        target_bir_lowering=False,
        debug=False,
        enable_asserts=True,
        # 3072-desc SWDGE ring so gather desc-gen runs ahead of DMA drain
        dynamic_dma_scratch_size=49152,
    )
    llrs_t = nc.dram_tensor("llrs_t", [NPAD, RP], BF16, kind="ExternalInput")
    idx_syn = nc.dram_tensor(
        "idx_syn", [128, NCHUNK * (EPC // 16)], I16, kind="ExternalInput"
    )
    idx_obs = nc.dram_tensor("idx_obs", [128, EOBS // 16], I16, kind="ExternalInput")
    sign_syn = nc.dram_tensor("sign_syn", [128, MPAD], BF16, kind="ExternalInput")
    sign_obs = nc.dram_tensor("sign_obs", [128, K], BF16, kind="ExternalInput")
    out = nc.dram_tensor("out", [1, 1], F32, kind="ExternalOutput")

    with tile.TileContext(nc) as tc:
        import contextlib

        with contextlib.ExitStack() as ctx:
            persist = ctx.enter_context(tc.tile_pool(name="persist", bufs=1))
            small = ctx.enter_context(tc.tile_pool(name="small", bufs=1))
            psum = ctx.enter_context(tc.tile_pool(name="ps", bufs=1, space="PSUM"))

            idx_syn_sb = persist.tile([128, NCHUNK * (EPC // 16)], I16)
            nc.sync.dma_start(idx_syn_sb[:], idx_syn[:])
            idx_obs_sb = persist.tile([128, EOBS // 16], I16)
            nc.sync.dma_start(idx_obs_sb[:], idx_obs[:])
            sign_syn_sb = persist.tile([128, MPAD], BF16)
            nc.sync.dma_start(sign_syn_sb[:], sign_syn[:])
            sign_obs_sb = persist.tile([128, K], BF16)
            nc.sync.dma_start(sign_obs_sb[:], sign_obs[:])

            # t (bf16) staged to DRAM so gathers read HBM (M2S) and write
            # SBUF (S2M) on separate paths instead of SBUF<->SBUF.
            dram = ctx.enter_context(tc.tile_pool(name="tdram", bufs=1,
                                                  space="DRAM"))
            t_dram = dram.tile([NPAD, RP], BF16)
            llr_src = llrs_t[:].rearrange("(a p) r -> p a r", p=128)
            t_dst = t_dram[:].rearrange("(a p) r -> p a r", p=128)
            with tc.tile_pool(name="llr_stage", bufs=2) as stage:
                for a0 in range(0, NRANKS, RANK_CHUNK):
                    a1 = min(a0 + RANK_CHUNK, NRANKS)
                    st = stage.tile([128, RANK_CHUNK, RP], BF16, tag="stage")
                    nc.sync.dma_start(
                        st[:, : a1 - a0, :], llr_src[:, a0:a1, :]
                    )
                    tt = stage.tile([128, RANK_CHUNK, RP], BF16, tag="tout")
                    nc.scalar.activation(
                        tt[:, : a1 - a0, :], st[:, : a1 - a0, :], AF.Tanh,
                        scale=0.5
                    )
                    nc.sync.dma_start(t_dst[:, a0:a1, :], tt[:, : a1 - a0, :])

            acc = small.tile([128, NCHUNK + 1], F32)

            with tc.tile_pool(name="g", bufs=2) as gpool:
                for c in range(nchunk):
                    # SWDGE descriptor ring holds 1023 descs/direction: a
                    # >1023-idx dma_gather wedges the Q7. Issue EPG=768-idx
                    # gathers, each into its own contiguous [:, k] block
                    # (transpose-mode output must be contiguous).
                    g = gpool.tile([128, GPC, FD, CHK_W * CPG], BF16, tag="G")
                    for k in range(GPC):
                        nc.gpsimd.dma_gather(
                            g[:, k],
                            t_dram[:],
                            idx_syn_sb[
                                :,
                                c * (EPC // 16) + k * (EPG // 16) : c * (EPC // 16)
                                + (k + 1) * (EPG // 16),
                            ],
                            EPG,
                            EPG,
                            RP,
                            transpose=True,
                        )
                    gv = g[:].rearrange("p k f (w c) -> p k f w c", w=CHK_W)
                    if not do_muls:
                        nc.scalar.activation(
                            gv[:, :, :, 0], gv[:, :, :, 0], AF.Ln, bias=1.0,
                            accum_out=acc[:, c:c + 1])
                        continue
                    nc.vector.tensor_mul(gv[:, :, :, 1], gv[:, :, :, 0], gv[:, :, :, 1])
                    nc.vector.tensor_mul(gv[:, :, :, 3], gv[:, :, :, 2], gv[:, :, :, 3])
                    nc.vector.tensor_mul(gv[:, :, :, 5], gv[:, :, :, 4], gv[:, :, :, 5])
                    nc.vector.tensor_mul(gv[:, :, :, 3], gv[:, :, :, 1], gv[:, :, :, 3])
                    sgn = sign_syn_sb[:, c * CPC : (c + 1) * CPC].rearrange(
                        "p (k c) -> p k c", k=GPC
                    )
                    for f in range(FD):
                        nc.vector.tensor_mul(gv[:, :, f, 5], gv[:, :, f, 5], sgn)
                    nc.vector.tensor_mul(gv[:, :, :, 0], gv[:, :, :, 3], gv[:, :, :, 5])
                    nc.scalar.activation(
                        gv[:, :, :, 0],
                        gv[:, :, :, 0],
                        AF.Ln,
                        bias=1.0,
                        accum_out=acc[:, c : c + 1],
                    )

                # observables: one chunk, halving tree over w (pads are 1.0)
                if do_obs:
                    g = gpool.tile([128, FD, EOBS], BF16, tag="G")
                    nc.gpsimd.dma_gather(
                        g[:],
                        t_dram[:],
                        idx_obs_sb[:],
                        EOBS,
                        EOBS,
                        RP,
                        transpose=True,
                    )
                    gv = g[:].rearrange("p f (w k) -> p f w k", w=OWPAD)
                    half = OWPAD // 2
                    while half >= 1:
                        nc.vector.tensor_mul(
                            gv[:, :, 0:half], gv[:, :, 0:half],
                            gv[:, :, half : 2 * half]
                        )
                        half //= 2
                    for f in range(FD):
                        nc.vector.tensor_mul(gv[:, f, 0], gv[:, f, 0],
                                             sign_obs_sb[:])
                    nc.scalar.activation(
                        gv[:, :, 0],
                        gv[:, :, 0],
                        AF.Ln,
                        bias=1.0,
                        accum_out=acc[:, NCHUNK : NCHUNK + 1],
                    )
                else:
                    nc.vector.memset(acc[:, NCHUNK : NCHUNK + 1], 0.0)

            if do_final:
                accsum = small.tile([128, 1], F32)
                nc.vector.tensor_reduce(
                    accsum[:], acc[:], mybir.AxisListType.X, mybir.AluOpType.add
                )
                ones = small.tile([128, 1], F32)
                nc.vector.memset(ones[:], 1.0)
                tot_ps = psum.tile([1, 1], F32)
                nc.tensor.matmul(tot_ps[:], ones[:], accsum[:])
                res = small.tile([1, 1], F32)
                cbias = small.tile([1, 1], F32)
                nc.vector.memset(cbias[:], _CONST)
                nc.scalar.activation(
                    res[:], tot_ps[:], AF.Identity, scale=_SCALE, bias=cbias[:]
                )
                nc.sync.dma_start(out[:], res[:])
            else:
                res = small.tile([1, 1], F32)
                nc.vector.tensor_reduce(
                    res[:], acc[:1, :], mybir.AxisListType.X, mybir.AluOpType.add
                )
                nc.sync.dma_start(out[:], res[:])

    nc.compile()
    return nc


def _segments(idx, seg, nseg, width):
    """Group entry column-indices by segment id -> [nseg, width] int array."""
    idx = np.asarray(idx, np.int64).ravel()
    seg = np.asarray(seg, np.int64).ravel()
    counts = np.bincount(seg, minlength=nseg)
    assert counts.max() <= width, (counts.max(), width)
    order = np.argsort(seg, kind="stable")
    out = np.full((nseg, width), ONES_COL, np.int64)
    srt = idx[order]
    pos = 0
    if (counts == width).all():
        out[:, :] = srt.reshape(nseg, width)
    else:
        for s in range(nseg):
            c = counts[s]
            out[s, :c] = srt[pos : pos + c]
            pos += c
    return out


def _wrap_idxs(flat):
    """[n] -> [128, n//16] int16, gather pos i = s*16 + (p%16)."""
    n = len(flat)
    s = np.arange(n // 16)
    out = np.empty((128, n // 16), np.int16)
    for p in range(128):
        out[p, :] = flat[s * 16 + (p % 16)]
    return out


def _prepare_in_maps(all_llrs, syndromes, observables, chk_idx, chk_seg,
                     obs_idx, obs_seg):
    all_llrs = np.asarray(all_llrs, np.float32)
    syndromes = np.asarray(syndromes)
    observables = np.asarray(observables)

    e6 = _segments(chk_idx, chk_seg, M, CHK_W)          # [M, 6]
    e6 = np.concatenate(
        [e6, np.full((MPAD - M, CHK_W), ONES_COL, np.int64)], axis=0
    )
    eo = _segments(obs_idx, obs_seg, K, OBS_W)          # [K, 50]
    eo = np.concatenate(
        [eo, np.full((K, OWPAD - OBS_W), ONES_COL, np.int64)], axis=1
    )

    # gather index tables (identical for every core)
    idx_syn = np.empty((128, NCHUNK * (EPC // 16)), np.int16)
    for c in range(NCHUNK):
        flat = np.empty(EPC, np.int64)
        for k in range(GPC):
            for w in range(CHK_W):
                lo = c * CPC + k * CPG
                flat[k * EPG + w * CPG : k * EPG + (w + 1) * CPG] = e6[
                    lo : lo + CPG, w
                ]
        idx_syn[:, c * (EPC // 16) : (c + 1) * (EPC // 16)] = _wrap_idxs(flat)
    flat = np.empty(EOBS, np.int64)
    for w in range(OWPAD):
        flat[w * K : (w + 1) * K] = eo[:, w]
    idx_obs = _wrap_idxs(flat)

    p16 = np.arange(128) % 16
    in_maps = []
    for core in range(NCORES):
        b0 = core * BC
        lt = np.zeros((NPAD, RP), np.float32)
        # row r = t*16 + b  (t-major)
        lt[:N, :R] = all_llrs[b0 : b0 + BC].transpose(2, 1, 0).reshape(N, R)
        lt[ONES_COL, :] = 88.0
        lt = lt.astype(ml_dtypes.bfloat16)
        ss = np.zeros((128, MPAD), np.float32)
        ss[:, :M] = 1.0 - 2.0 * syndromes[b0 + p16, :].astype(np.float32)
        so = 1.0 - 2.0 * observables[b0 + p16, :].astype(np.float32)
        in_maps.append(
            {
                "llrs_t": lt,
                "idx_syn": idx_syn,
                "idx_obs": idx_obs,
                "sign_syn": ss.astype(ml_dtypes.bfloat16),
                "sign_obs": so.astype(ml_dtypes.bfloat16),
            }
        )
    return in_maps


def kernel(all_llrs, syndromes, observables, chk_idx, chk_seg, obs_idx, obs_seg):
    global _nc_cache, LAST_RESULTS
    in_maps = _prepare_in_maps(
        all_llrs, syndromes, observables, chk_idx, chk_seg, obs_idx, obs_seg
    )

    if _nc_cache is None:
        _nc_cache = _build_nc()

    trace = os.environ.get("KERNEL_TRACE", "") == "1"
    LAST_RESULTS = run_bass_kernel_spmd(
        _nc_cache, in_maps, core_ids=list(range(NCORES)), trace=trace
    )
    total = sum(float(r["out"][0, 0]) for r in LAST_RESULTS.results)
    return np.float32(total)
